# revision 19
# baseline (speedup 1.0000x reference)
"""Trainium2 Bass kernel for nn_CaptionModel (GRU caption decoder).

Math (per reference):
  h0 = feat @ w_hp + b_hp                      [B, H]
  x0 = embed[SOS]  (broadcast over batch)
  for t in 1..200:  h_t = GRUCell(x_{t-1}, h_{t-1})  with x_t = h_t
  out[b, v, t] = (h_t @ w_proj + b_proj)[b, v]

Key algebra: for t >= 2 the GRU input x equals h, so the r/z gates fold into
a combined weight W'_r = w_ih_r + w_hh_r (same for z); the n gate keeps
w_ih_n / w_hh_n separate (r multiplies only the h-side):
  pre = h @ W'.T,  W' = [W'_r; W'_z; w_ih_n; w_hh_n]   [2048, 512]
  r = sig(pre_r), z = sig(pre_z), n = tanh(pre_in + r * pre_hn)
  h' = n + z*(h - n)
Step 1 input x0 is batch-constant: g0 = w_ih @ embed[SOS] + b_ih folds into
per-partition activation biases.

Device layout (per core, batch slice Bc=64, pure data parallel over 8 cores):
  Everything transposed: hT [H=512 -> 4 partition-chunks of 128, Bc free].
  Per H-chunk c the four gate pre-act M-tiles [128, Bc] land in two PSUM
  tiles: gA = [r_c | z_c], gB = [in_c | hn_c]; chunks alternate PSUM slots so
  the PE never serializes against elementwise readers of the previous chunk.

Perf structure: the step is LDWEIGHTS-bound (64 stationary weight tiles per
step; the full W' must pass through the PE array every step since the PE
holds only 2 weight buffers). Two optimizations over the plain-bf16 version:
  1. The combined gate weights wT are stored fp8-e4m3 (moving h stays bf16 -
     mixed-dtype matmul). FWL reads 4 fp8/cycle vs 2 bf16/cycle, halving
     LDWEIGHTS time: ~53ns -> ~27ns per [128,128] tile, ~5.1us -> ~3.9us per
     step modeled. Accuracy: sigmoid/tanh damp the ~3.6% RMS weight
     quantization; measured on device: max rel err 0.0094 (gate 2e-2;
     numpy/CoreSim predicted 0.0084-0.0088).
  2. The h' = n + z*(h-n) tail is computed as (1-z)*n + z*h with u = z*h and
     w = 1-z precomputed on the DVE off the critical chain right after the
     z-sigmoid (NOT on gpsimd/POOL: its tensor ops silently corrupt on this
     runtime), and the sigmoid is split r-first so the critical
     t1 = r*hn -> tanh chain starts ~110ns earlier. CoreSim non-PE critical
     path: 3.64 -> 3.26 us/step; the remaining per-step PE stall waiting on
     the elementwise chain is ~1.1us (trace: Matmult -> next step's Ldweights,
     which carries the matmul's moving-operand wait).
  3. logits are kept t-major [Bc, steps, V] and streamed to HBM in 4-step
     batches during compute instead of one 5MB end-of-kernel DMA (CoreSim:
     -30us of unhidden tail).
Step-1 (x0 = embed[SOS]) weights whhT and the h0 projection stay bf16.
Rejected with evidence: proj deferral by one step (scheduler does better with
program order), gate-MM emission reorders (ditto, -370ns/step), gpsimd
offload (HW corruption), remote_dma TP (device faults), flipped h-stationary
layout (see kernel2.py: all-to-all contraction exposes the full chain; sims
5.4+ us/step), fp8 w_proj (numpy end-to-end rel err 0.035 > 2e-2 gate: the
output projection has no sigmoid/tanh damping), fp8 w_hp (h0 error injected
undamped into the recurrence).
"""

import numpy as np
from contextlib import ExitStack

import concourse.bass as bass
import concourse.bacc as bacc
import concourse.mybir as mybir
import concourse.tile as tile
from concourse.bass_utils import run_bass_kernel_spmd

B, FEAT, H, V = 512, 2048, 512, 100
STEPS = 200
SOS = 0
NCORES = 8
Bc = B // NCORES           # 64 batch rows per core
KC = H // 128              # 4 contraction chunks over H
KF = FEAT // 128           # 16 contraction chunks over FEAT
F32 = mybir.dt.float32
BF16 = mybir.dt.bfloat16
FP8 = mybir.dt.float8e4
AF = mybir.ActivationFunctionType
OP = mybir.AluOpType

BF16_NP = mybir.dt.np(BF16)
FP8_NP = mybir.dt.np(FP8)

LAST_RESULTS = None        # test harness introspection (profile/timing)

_PROGRAM_CACHE = {}


def _build(nc_biases, steps=STEPS, reps=1, mode="full"):
    """Build the Bass program. nc_biases: frozenset of nonzero bias groups in
    {"rz", "hn", "in", "hp", "proj"} (grading inputs are all-zero biases, so
    the hot path emits no bias work beyond the step-1 g0 fold)."""
    nc = bacc.Bacc(debug=False)

    wT_d = nc.dram_tensor("wT", [KC, 128, 4 * H], FP8, kind="ExternalInput")
    whhT_d = nc.dram_tensor("whhT", [KC, 128, 3 * H], BF16, kind="ExternalInput")
    whpT_d = nc.dram_tensor("whpT", [KF, 128, H], BF16, kind="ExternalInput")
    featT_d = nc.dram_tensor("featT", [KF, 128, Bc], BF16, kind="ExternalInput")
    wproj_d = nc.dram_tensor("wproj", [KC, 128, 128], BF16, kind="ExternalInput")
    # Step-1 activation biases (g0 folded; always present), layout [128, KC]:
    # column c is the [128,1] per-partition bias for H-chunk c.
    b1r_d = nc.dram_tensor("b1r", [128, KC], F32, kind="ExternalInput")
    b1z_d = nc.dram_tensor("b1z", [128, KC], F32, kind="ExternalInput")
    b1n_d = nc.dram_tensor("b1n", [128, KC], F32, kind="ExternalInput")
    has_rz = "rz" in nc_biases
    has_hn = "hn" in nc_biases
    has_in = "in" in nc_biases
    has_hp = "hp" in nc_biases
    has_proj = "proj" in nc_biases
    optd = {}
    for name, present in (("br", has_rz), ("bz", has_rz), ("bhn", has_hn),
                          ("bin", has_in), ("bhp", has_hp)):
        if present:
            optd[name] = nc.dram_tensor(name, [128, KC], F32, kind="ExternalInput")
    if has_proj:
        bproj_d = nc.dram_tensor("bproj", [V, Bc], F32, kind="ExternalInput")
    out_d = nc.dram_tensor("out", [V, steps, Bc], F32, kind="ExternalOutput")

    with tile.TileContext(nc) as tc, ExitStack() as ctx:
        const = ctx.enter_context(tc.tile_pool(name="const", bufs=1))
        hpool = ctx.enter_context(tc.tile_pool(name="h", bufs=4))
        ew = ctx.enter_context(tc.tile_pool(name="ew", bufs=6))
        psum = ctx.enter_context(
            tc.tile_pool(name="psum", bufs=2, space=bass.MemorySpace.PSUM)
        )

        # ---- constants into SBUF ----
        wT = const.tile([128, KC, 4 * H], FP8)
        whhT = const.tile([128, KC, 3 * H], BF16)
        whpT = const.tile([128, KF, H], BF16)
        featT = const.tile([128, KF, Bc], BF16)
        wproj = const.tile([128, KC, 128], BF16)
        for k in range(KC):
            nc.sync.dma_start(wT[:, k, :], wT_d[k])
            nc.sync.dma_start(whhT[:, k, :], whhT_d[k])
            nc.sync.dma_start(wproj[:, k, :], wproj_d[k])
        for k in range(KF):
            nc.sync.dma_start(whpT[:, k, :], whpT_d[k])
            nc.sync.dma_start(featT[:, k, :], featT_d[k])
        b1r = const.tile([128, KC], F32)
        b1z = const.tile([128, KC], F32)
        b1n = const.tile([128, KC], F32)
        nc.sync.dma_start(b1r[:], b1r_d[:])
        nc.sync.dma_start(b1z[:], b1z_d[:])
        nc.sync.dma_start(b1n[:], b1n_d[:])
        opt = {}
        for name, d in optd.items():
            t = const.tile([128, KC], F32)
            nc.sync.dma_start(t[:], d[:])
            opt[name] = t
        if has_proj:
            bproj = const.tile([V, Bc], F32)
            nc.sync.dma_start(bproj[:], bproj_d[:])

        logits = const.tile([128, steps, Bc], F32)
        if mode in ("mm", "noproj", "chain_dve", "chain_mix", "chain_act", "ew2x"):
            # timing-only modes skip proj; logits must still be written once
            nc.gpsimd.memset(logits[:], 0.0)

        # ---- h0 = feat @ w_hp (+ b_hp), produced directly as hT chunks ----
        hbf_cur = hpool.tile([128, KC * Bc], BF16, tag="hbf")
        for m in range(KC):
            h0ps = psum.tile([128, Bc], F32, tag="gA", bufs=3)
            for k in range(KF):
                nc.tensor.matmul(
                    h0ps[:],
                    whpT[:, k, m * 128:(m + 1) * 128],
                    featT[:, k, :],
                    start=(k == 0), stop=(k == KF - 1),
                )
            sl = slice(m * Bc, (m + 1) * Bc)
            if has_hp:
                nc.vector.tensor_scalar_add(hbf_cur[:, sl], h0ps[:],
                                            opt["bhp"][:, m:m + 1])
            else:
                nc.vector.tensor_copy(hbf_cur[:, sl], h0ps[:])

        # ---- recurrence ----
        # PSUM halves: per H-half hf (chunks 2hf, 2hf+1), gA = [r_c0 r_c1 |
        # z_c0 z_c1], gB = [in_c0 in_c1 | hn_c0 hn_c1]; elementwise runs at
        # [128, 2*Bc] granularity on the zero-bias fast path.
        def emit_half_mms(first, hf, gA, gB, rhs):
            # NOTE: emission order (r, z, in, hn) measured best in CoreSim;
            # "chain-friendly" reorders like (r, hn, in, z) regressed the
            # scheduler's cross-step overlap by ~370ns/step.
            if first:
                gates = ((gA, 0, 0), (gA, 2 * Bc, H), (gB, 2 * Bc, 2 * H))
                wsrc = whhT
            else:
                gates = ((gA, 0, 0), (gA, 2 * Bc, H),
                         (gB, 0, 2 * H), (gB, 2 * Bc, 3 * H))
                wsrc = wT
            for bank, boff, gcol in gates:
                for ci in range(2):
                    dst = bank[:, boff + ci * Bc: boff + (ci + 1) * Bc]
                    m0 = gcol + (2 * hf + ci) * 128
                    for k in range(KC):
                        nc.tensor.matmul(
                            dst, wsrc[:, k, m0:m0 + 128],
                            rhs[:, k * Bc:(k + 1) * Bc],
                            start=(k == 0), stop=(k == KC - 1),
                        )

        fast = not (has_rz or has_hn or has_in)

        def gru_step(t, hbf_prev):
            first = (t == 1)
            hbf_next = hpool.tile([128, KC * Bc], BF16, tag="hbf")
            for hf in range(2):
                gA = psum.tile([128, 4 * Bc], F32, tag="gA", bufs=3)
                gB = psum.tile([128, 4 * Bc], F32, tag="gB", bufs=3)
                emit_half_mms(first, hf, gA, gB, hbf_prev)
                hsl = slice(hf * 2 * Bc, (hf + 1) * 2 * Bc)
                if fast and not first:
                    rz = ew.tile([128, 4 * Bc], BF16, tag="rz")
                    r2, z2 = rz[:, 0:2 * Bc], rz[:, 2 * Bc:4 * Bc]
                    t1 = ew.tile([128, 2 * Bc], BF16, tag="t1")
                    t2 = ew.tile([128, 2 * Bc], BF16, tag="t2")
                    n2 = ew.tile([128, 2 * Bc], BF16, tag="n")
                    u2 = ew.tile([128, 2 * Bc], BF16, tag="u")
                    w2 = ew.tile([128, 2 * Bc], BF16, tag="w")
                    e2 = ew.tile([128, 2 * Bc], BF16, tag="e")
                    # r-half first: the critical chain needs only r for
                    # t1 = r*hn; z feeds the off-chain u/w ops
                    nc.scalar.activation(rz[:, 0:2 * Bc], gA[:, 0:2 * Bc],
                                         AF.Sigmoid)
                    nc.scalar.activation(rz[:, 2 * Bc:4 * Bc],
                                         gA[:, 2 * Bc:4 * Bc], AF.Sigmoid)
                    # off the critical chain (DVE, z-dependent only):
                    # u = z*h, w = 1-z  (NOT on gpsimd/POOL: its tensor ops
                    # silently corrupt on this runtime - sim-only support)
                    nc.vector.tensor_mul(u2[:], z2, hbf_prev[:, hsl])
                    nc.vector.tensor_scalar(w2[:], z2, -1.0, 1.0,
                                            OP.mult, OP.add)
                    nc.vector.tensor_mul(t1[:], r2, gB[:, 2 * Bc:4 * Bc])
                    nc.vector.tensor_add(t2[:], t1[:], gB[:, 0:2 * Bc])
                    nc.scalar.activation(n2[:], t2[:], AF.Tanh)
                    # h' = (1-z)*n + z*h
                    nc.vector.tensor_mul(e2[:], n2[:], w2[:])
                    nc.vector.tensor_add(hbf_next[:, hsl], e2[:], u2[:])
                    continue
                # bias path (step 1 / nonzero biases): per-chunk, per-partition
                # biases differ per chunk so activations stay [128, Bc]
                for ci in range(2):
                    c = 2 * hf + ci
                    csl = slice(c * Bc, (c + 1) * Bc)
                    cc = slice(c, c + 1)
                    rps = gA[:, ci * Bc:(ci + 1) * Bc]
                    zps = gA[:, 2 * Bc + ci * Bc: 2 * Bc + (ci + 1) * Bc]
                    inps = gB[:, ci * Bc:(ci + 1) * Bc]
                    hnps = gB[:, 2 * Bc + ci * Bc: 2 * Bc + (ci + 1) * Bc]
                    r = ew.tile([128, Bc], BF16, tag="r")
                    z = ew.tile([128, Bc], BF16, tag="z")
                    t1 = ew.tile([128, Bc], BF16, tag="t1")
                    n = ew.tile([128, Bc], BF16, tag="n")
                    d = ew.tile([128, Bc], BF16, tag="d")
                    e = ew.tile([128, Bc], BF16, tag="e")
                    if first:
                        nc.scalar.activation(r[:], rps, AF.Sigmoid, bias=b1r[:, cc])
                        nc.scalar.activation(z[:], zps, AF.Sigmoid, bias=b1z[:, cc])
                    elif has_rz:
                        nc.scalar.activation(r[:], rps, AF.Sigmoid,
                                             bias=opt["br"][:, cc])
                        nc.scalar.activation(z[:], zps, AF.Sigmoid,
                                             bias=opt["bz"][:, cc])
                    else:
                        nc.scalar.activation(r[:], rps, AF.Sigmoid)
                        nc.scalar.activation(z[:], zps, AF.Sigmoid)
                    if has_hn:
                        nc.vector.scalar_tensor_tensor(t1[:], hnps,
                                                       opt["bhn"][:, cc],
                                                       r[:], OP.add, OP.mult)
                    else:
                        nc.vector.tensor_mul(t1[:], r[:], hnps)
                    if first:
                        nc.scalar.activation(n[:], t1[:], AF.Tanh, bias=b1n[:, cc])
                    else:
                        t2 = ew.tile([128, Bc], BF16, tag="t2")
                        nc.vector.tensor_add(t2[:], t1[:], inps)
                        if has_in:
                            nc.scalar.activation(n[:], t2[:], AF.Tanh,
                                                 bias=opt["bin"][:, cc])
                        else:
                            nc.scalar.activation(n[:], t2[:], AF.Tanh)
                    nc.vector.scalar_tensor_tensor(d[:], n[:], -1.0,
                                                   hbf_prev[:, csl],
                                                   OP.mult, OP.add)
                    nc.vector.tensor_mul(e[:], z[:], d[:])
                    nc.vector.tensor_add(hbf_next[:, csl], n[:], e[:])
            return hbf_next

        def proj_step(t, hbf):
            # wproj is the stationary operand (128-col zero-padded -> FWL);
            # output lands V-major [V(pad 128), Bc]; host de-transposes.
            pj = psum.tile([128, Bc], F32, tag="proj", bufs=2)
            for k in range(KC):
                nc.tensor.matmul(pj[:], wproj[:, k, :],
                                 hbf[:, k * Bc:(k + 1) * Bc],
                                 start=(k == 0), stop=(k == KC - 1))
            if has_proj:
                nc.vector.tensor_add(logits[0:V, t - 1, :], pj[0:V, :], bproj[:])
            else:
                nc.scalar.copy(logits[0:V, t - 1, :], pj[0:V, :])

        def gru_step_mm(t):
            first = (t == 1)
            for hf in range(2):
                gA = psum.tile([128, 4 * Bc], F32, tag="gA", bufs=3)
                gB = psum.tile([128, 4 * Bc], F32, tag="gB", bufs=3)
                emit_half_mms(first, hf, gA, gB, hbf_cur)

        if mode.startswith("chain"):
            # dependency-chain microbenchmarks: each "step" = 10 dependent ops
            ca = ew.tile([128, Bc], BF16, tag="ca")
            cb = ew.tile([128, Bc], BF16, tag="cb")
            nc.vector.tensor_add(ca[:], featT[:, 0, :], featT[:, 1, :])
            nc.vector.tensor_add(cb[:], featT[:, 1, :], featT[:, 2, :])
            acc = ca
            for t in range(steps * reps):
                for i in range(10):
                    nxt = ew.tile([128, Bc], BF16, tag=f"cc{i % 4}")
                    if mode == "chain_dve" or (mode == "chain_mix" and i % 2 == 0):
                        nc.vector.tensor_add(nxt[:], acc[:], cb[:])
                    else:
                        nc.scalar.activation(nxt[:], acc[:], AF.Sigmoid)
                    acc = nxt
            nc.vector.tensor_add(logits[0:Bc, 0, 0:Bc], acc[0:Bc, 0:Bc],
                                 acc[0:Bc, 0:Bc])
            nc.sync.dma_start(out_d[:], logits[:])
            nc.compile()
            return nc

        for rep in range(reps):
            for t in range(1, steps + 1):
                if mode == "mm":
                    gru_step_mm(t)
                elif mode == "mmproj":
                    gru_step_mm(t)
                    proj_step(t, hbf_cur)
                elif mode == "noproj":
                    hbf_cur = gru_step(t, hbf_cur)
                else:
                    hbf_cur = gru_step(t, hbf_cur)
                    proj_step(t, hbf_cur)
                if mode == "full" and t % 4 == 0 and rep == reps - 1:
                    # stream logits out in 4-step batches (contiguous runs),
                    # hidden under compute instead of a ~28us end tail
                    nc.sync.dma_start(out_d[:, t - 4:t, :],
                                      logits[0:V, t - 4:t, :])
        if mode == "full":
            if steps % 4 != 0:
                t0 = steps - steps % 4
                nc.sync.dma_start(out_d[:, t0:steps, :], logits[0:V, t0:steps, :])
        else:
            nc.sync.dma_start(out_d[:], logits[0:V])

    nc.compile()
    return nc


def _prep_inputs(feat, w_hp, b_hp, embed, w_ih, w_hh, b_ih, b_hh, w_proj, b_proj):
    f32 = np.float32
    feat = np.asarray(feat, f32)
    w_hp = np.asarray(w_hp, f32)
    b_hp = np.asarray(b_hp, f32)
    embed = np.asarray(embed, f32)
    w_ih = np.asarray(w_ih, f32)
    w_hh = np.asarray(w_hh, f32)
    b_ih = np.asarray(b_ih, f32)
    b_hh = np.asarray(b_hh, f32)
    w_proj = np.asarray(w_proj, f32)
    b_proj = np.asarray(b_proj, f32)

    def chunk_bias(v):          # [H] -> [128, KC] (col c = chunk c)
        return np.ascontiguousarray(v.reshape(KC, 128).T.astype(f32))

    Wc = np.concatenate([
        w_ih[0:H] + w_hh[0:H],
        w_ih[H:2 * H] + w_hh[H:2 * H],
        w_ih[2 * H:3 * H],
        w_hh[2 * H:3 * H],
    ], axis=0)                                   # [4H, H]
    wT = np.ascontiguousarray(Wc.T.reshape(KC, 128, 4 * H).astype(FP8_NP))
    whhT = np.ascontiguousarray(w_hh.T.reshape(KC, 128, 3 * H).astype(BF16_NP))
    whpT = np.ascontiguousarray(w_hp.reshape(KF, 128, H).astype(BF16_NP))
    wproj_pad = np.zeros((H, 128), f32)
    wproj_pad[:, 0:V] = w_proj
    wproj = np.ascontiguousarray(wproj_pad.reshape(KC, 128, 128).astype(BF16_NP))

    g0 = w_ih @ embed[SOS] + b_ih               # [3H]
    common = dict(wT=wT, whhT=whhT, whpT=whpT, wproj=wproj,
                  b1r=chunk_bias(g0[0:H] + b_hh[0:H]),
                  b1z=chunk_bias(g0[H:2 * H] + b_hh[H:2 * H]),
                  b1n=chunk_bias(g0[2 * H:3 * H]))

    biases = set()
    if np.any(b_ih[0:2 * H] + b_hh[0:2 * H]):
        biases.add("rz")
        common["br"] = chunk_bias(b_ih[0:H] + b_hh[0:H])
        common["bz"] = chunk_bias(b_ih[H:2 * H] + b_hh[H:2 * H])
    if np.any(b_hh[2 * H:]):
        biases.add("hn")
        common["bhn"] = chunk_bias(b_hh[2 * H:])
    if np.any(b_ih[2 * H:]):
        biases.add("in")
        common["bin"] = chunk_bias(b_ih[2 * H:])
    if np.any(b_hp):
        biases.add("hp")
        common["bhp"] = chunk_bias(b_hp)
    if np.any(b_proj):
        biases.add("proj")
        common["bproj"] = np.ascontiguousarray(
            np.broadcast_to(b_proj[:, None], (V, Bc)).astype(f32))

    featT = feat.T.astype(BF16_NP)               # [FEAT, B]
    in_maps = []
    for c in range(NCORES):
        m = dict(common)
        m["featT"] = np.ascontiguousarray(
            featT[:, c * Bc:(c + 1) * Bc].reshape(KF, 128, Bc))
        in_maps.append(m)
    return frozenset(biases), in_maps


def kernel(**inputs) -> np.ndarray:
    global LAST_RESULTS
    biases, in_maps = _prep_inputs(**inputs)
    if biases not in _PROGRAM_CACHE:
        _PROGRAM_CACHE[biases] = _build(biases)
    nc = _PROGRAM_CACHE[biases]
    res = run_bass_kernel_spmd(nc, in_maps, list(range(NCORES)))
    LAST_RESULTS = res
    outs = [res.results[c]["out"].transpose(2, 0, 1) for c in range(NCORES)]
    out = np.concatenate(outs, axis=0)
    return np.ascontiguousarray(out)



# revision 20
# speedup vs baseline: 1.0199x; 1.0199x over previous
"""Trainium2 Bass kernel for nn_CaptionModel (GRU caption decoder).

Math (per reference):
  h0 = feat @ w_hp + b_hp                      [B, H]
  x0 = embed[SOS]  (broadcast over batch)
  for t in 1..200:  h_t = GRUCell(x_{t-1}, h_{t-1})  with x_t = h_t
  out[b, v, t] = (h_t @ w_proj + b_proj)[b, v]

Key algebra: for t >= 2 the GRU input x equals h, so the r/z gates fold into
a combined weight W'_r = w_ih_r + w_hh_r (same for z); the n gate keeps
w_ih_n / w_hh_n separate (r multiplies only the h-side):
  pre = h @ W'.T,  W' = [W'_r; W'_z; w_ih_n; w_hh_n]   [2048, 512]
  r = sig(pre_r), z = sig(pre_z), n = tanh(pre_in + r * pre_hn)
  h' = n + z*(h - n)
Step 1 input x0 is batch-constant: g0 = w_ih @ embed[SOS] + b_ih folds into
per-partition activation biases.

Device layout (per core, batch slice Bc=64, pure data parallel over 8 cores):
  Everything transposed: hT [H=512 -> 4 partition-chunks of 128, Bc free].
  Per H-chunk c the four gate pre-act M-tiles [128, Bc] land in two PSUM
  tiles: gA = [r_c | z_c], gB = [in_c | hn_c]; chunks alternate PSUM slots so
  the PE never serializes against elementwise readers of the previous chunk.

Perf structure: the step is LDWEIGHTS-bound (64 stationary weight tiles per
step; the full W' must pass through the PE array every step since the PE
holds only 2 weight buffers). Two optimizations over the plain-bf16 version:
  1. The combined gate weights wT are stored fp8-e4m3 (moving h stays bf16 -
     mixed-dtype matmul). FWL reads 4 fp8/cycle vs 2 bf16/cycle, halving
     LDWEIGHTS time: ~53ns -> ~27ns per [128,128] tile, ~5.1us -> ~3.9us per
     step modeled. Accuracy: sigmoid/tanh damp the ~3.6% RMS weight
     quantization; measured on device: max rel err 0.0094 (gate 2e-2;
     numpy/CoreSim predicted 0.0084-0.0088).
  2. The h' = n + z*(h-n) tail is computed as (1-z)*n + z*h with u = z*h and
     w = 1-z precomputed on the DVE off the critical chain right after the
     z-sigmoid (NOT on gpsimd/POOL: its tensor ops silently corrupt on this
     runtime), and the sigmoid is split r-first so the critical
     t1 = r*hn -> tanh chain starts ~110ns earlier. CoreSim non-PE critical
     path: 3.64 -> 3.26 us/step; the remaining per-step PE stall waiting on
     the elementwise chain is ~1.1us (trace: Matmult -> next step's Ldweights,
     which carries the matmul's moving-operand wait).
  3. logits are kept t-major and streamed to HBM in 4-step batches during
     compute instead of one 5MB end-of-kernel DMA (CoreSim: -30us of
     unhidden tail).
  4. the projection runs with wproj as the stationary operand (zero-padded
     to 128 cols -> FWL-eligible bf16, LDW 53 vs 97ns pairs), producing
     V-major logits [V, steps, Bc] that the host de-transposes for free
     (CoreSim slope 3258 -> 3074 ns/step).
Step-1 (x0 = embed[SOS]) weights whhT and the h0 projection stay bf16.
Rejected with evidence: proj deferral by one step (scheduler does better with
program order), gate-MM emission reorders (ditto, -370ns/step), gpsimd
offload (HW corruption), remote_dma TP (device faults), flipped h-stationary
layout (see kernel2.py: all-to-all contraction exposes the full chain; sims
5.4+ us/step), fp8 w_proj (numpy end-to-end rel err 0.035 > 2e-2 gate: the
output projection has no sigmoid/tanh damping), fp8 w_hp (h0 error injected
undamped into the recurrence).
"""

import numpy as np
from contextlib import ExitStack

import concourse.bass as bass
import concourse.bacc as bacc
import concourse.mybir as mybir
import concourse.tile as tile
from concourse.bass_utils import run_bass_kernel_spmd

B, FEAT, H, V = 512, 2048, 512, 100
STEPS = 200
SOS = 0
NCORES = 8
Bc = B // NCORES           # 64 batch rows per core
KC = H // 128              # 4 contraction chunks over H
KF = FEAT // 128           # 16 contraction chunks over FEAT
F32 = mybir.dt.float32
BF16 = mybir.dt.bfloat16
FP8 = mybir.dt.float8e4
AF = mybir.ActivationFunctionType
OP = mybir.AluOpType

BF16_NP = mybir.dt.np(BF16)
FP8_NP = mybir.dt.np(FP8)

LAST_RESULTS = None        # test harness introspection (profile/timing)

_PROGRAM_CACHE = {}


def _build(nc_biases, steps=STEPS, reps=1, mode="full"):
    """Build the Bass program. nc_biases: frozenset of nonzero bias groups in
    {"rz", "hn", "in", "hp", "proj"} (grading inputs are all-zero biases, so
    the hot path emits no bias work beyond the step-1 g0 fold)."""
    nc = bacc.Bacc(debug=False)

    wT_d = nc.dram_tensor("wT", [KC, 128, 4 * H], FP8, kind="ExternalInput")
    whhT_d = nc.dram_tensor("whhT", [KC, 128, 3 * H], BF16, kind="ExternalInput")
    whpT_d = nc.dram_tensor("whpT", [KF, 128, H], BF16, kind="ExternalInput")
    featT_d = nc.dram_tensor("featT", [KF, 128, Bc], BF16, kind="ExternalInput")
    wproj_d = nc.dram_tensor("wproj", [KC, 128, 128], BF16, kind="ExternalInput")
    # Step-1 activation biases (g0 folded; always present), layout [128, KC]:
    # column c is the [128,1] per-partition bias for H-chunk c.
    b1r_d = nc.dram_tensor("b1r", [128, KC], F32, kind="ExternalInput")
    b1z_d = nc.dram_tensor("b1z", [128, KC], F32, kind="ExternalInput")
    b1n_d = nc.dram_tensor("b1n", [128, KC], F32, kind="ExternalInput")
    has_rz = "rz" in nc_biases
    has_hn = "hn" in nc_biases
    has_in = "in" in nc_biases
    has_hp = "hp" in nc_biases
    has_proj = "proj" in nc_biases
    optd = {}
    for name, present in (("br", has_rz), ("bz", has_rz), ("bhn", has_hn),
                          ("bin", has_in), ("bhp", has_hp)):
        if present:
            optd[name] = nc.dram_tensor(name, [128, KC], F32, kind="ExternalInput")
    if has_proj:
        bproj_d = nc.dram_tensor("bproj", [V, Bc], F32, kind="ExternalInput")
    out_d = nc.dram_tensor("out", [V, steps, Bc], F32, kind="ExternalOutput")

    with tile.TileContext(nc) as tc, ExitStack() as ctx:
        const = ctx.enter_context(tc.tile_pool(name="const", bufs=1))
        hpool = ctx.enter_context(tc.tile_pool(name="h", bufs=4))
        ew = ctx.enter_context(tc.tile_pool(name="ew", bufs=6))
        psum = ctx.enter_context(
            tc.tile_pool(name="psum", bufs=2, space=bass.MemorySpace.PSUM)
        )

        # ---- constants into SBUF ----
        wT = const.tile([128, KC, 4 * H], FP8)
        whhT = const.tile([128, KC, 3 * H], BF16)
        whpT = const.tile([128, KF, H], BF16)
        featT = const.tile([128, KF, Bc], BF16)
        wproj = const.tile([128, KC, 128], BF16)
        for k in range(KC):
            nc.sync.dma_start(wT[:, k, :], wT_d[k])
            nc.sync.dma_start(whhT[:, k, :], whhT_d[k])
            nc.sync.dma_start(wproj[:, k, :], wproj_d[k])
        for k in range(KF):
            nc.sync.dma_start(whpT[:, k, :], whpT_d[k])
            nc.sync.dma_start(featT[:, k, :], featT_d[k])
        b1r = const.tile([128, KC], F32)
        b1z = const.tile([128, KC], F32)
        b1n = const.tile([128, KC], F32)
        nc.sync.dma_start(b1r[:], b1r_d[:])
        nc.sync.dma_start(b1z[:], b1z_d[:])
        nc.sync.dma_start(b1n[:], b1n_d[:])
        opt = {}
        for name, d in optd.items():
            t = const.tile([128, KC], F32)
            nc.sync.dma_start(t[:], d[:])
            opt[name] = t
        if has_proj:
            bproj = const.tile([V, Bc], F32)
            nc.sync.dma_start(bproj[:], bproj_d[:])

        logits = const.tile([128, steps, Bc], F32)
        if mode in ("mm", "noproj", "chain_dve", "chain_mix", "chain_act", "ew2x"):
            # timing-only modes skip proj; logits must still be written once
            nc.gpsimd.memset(logits[:], 0.0)

        # ---- h0 = feat @ w_hp (+ b_hp), produced directly as hT chunks ----
        hbf_cur = hpool.tile([128, KC * Bc], BF16, tag="hbf")
        for m in range(KC):
            h0ps = psum.tile([128, Bc], F32, tag="gA", bufs=3)
            for k in range(KF):
                nc.tensor.matmul(
                    h0ps[:],
                    whpT[:, k, m * 128:(m + 1) * 128],
                    featT[:, k, :],
                    start=(k == 0), stop=(k == KF - 1),
                )
            sl = slice(m * Bc, (m + 1) * Bc)
            if has_hp:
                nc.vector.tensor_scalar_add(hbf_cur[:, sl], h0ps[:],
                                            opt["bhp"][:, m:m + 1])
            else:
                nc.vector.tensor_copy(hbf_cur[:, sl], h0ps[:])

        # ---- recurrence ----
        # PSUM halves: per H-half hf (chunks 2hf, 2hf+1), gA = [r_c0 r_c1 |
        # z_c0 z_c1], gB = [in_c0 in_c1 | hn_c0 hn_c1]; elementwise runs at
        # [128, 2*Bc] granularity on the zero-bias fast path.
        def emit_half_mms(first, hf, gA, gB, rhs):
            # NOTE: emission order (r, z, in, hn) measured best in CoreSim;
            # "chain-friendly" reorders like (r, hn, in, z) regressed the
            # scheduler's cross-step overlap by ~370ns/step.
            if first:
                gates = ((gA, 0, 0), (gA, 2 * Bc, H), (gB, 2 * Bc, 2 * H))
                wsrc = whhT
            else:
                gates = ((gA, 0, 0), (gA, 2 * Bc, H),
                         (gB, 0, 2 * H), (gB, 2 * Bc, 3 * H))
                wsrc = wT
            for bank, boff, gcol in gates:
                for ci in range(2):
                    dst = bank[:, boff + ci * Bc: boff + (ci + 1) * Bc]
                    m0 = gcol + (2 * hf + ci) * 128
                    for k in range(KC):
                        nc.tensor.matmul(
                            dst, wsrc[:, k, m0:m0 + 128],
                            rhs[:, k * Bc:(k + 1) * Bc],
                            start=(k == 0), stop=(k == KC - 1),
                        )

        fast = not (has_rz or has_hn or has_in)

        def gru_step(t, hbf_prev):
            first = (t == 1)
            hbf_next = hpool.tile([128, KC * Bc], BF16, tag="hbf")
            for hf in range(2):
                gA = psum.tile([128, 4 * Bc], F32, tag="gA", bufs=3)
                gB = psum.tile([128, 4 * Bc], F32, tag="gB", bufs=3)
                emit_half_mms(first, hf, gA, gB, hbf_prev)
                hsl = slice(hf * 2 * Bc, (hf + 1) * 2 * Bc)
                if fast and not first:
                    rz = ew.tile([128, 4 * Bc], BF16, tag="rz")
                    r2, z2 = rz[:, 0:2 * Bc], rz[:, 2 * Bc:4 * Bc]
                    t1 = ew.tile([128, 2 * Bc], BF16, tag="t1")
                    t2 = ew.tile([128, 2 * Bc], BF16, tag="t2")
                    n2 = ew.tile([128, 2 * Bc], BF16, tag="n")
                    u2 = ew.tile([128, 2 * Bc], BF16, tag="u")
                    w2 = ew.tile([128, 2 * Bc], BF16, tag="w")
                    e2 = ew.tile([128, 2 * Bc], BF16, tag="e")
                    # r-half first: the critical chain needs only r for
                    # t1 = r*hn; z feeds the off-chain u/w ops
                    nc.scalar.activation(rz[:, 0:2 * Bc], gA[:, 0:2 * Bc],
                                         AF.Sigmoid)
                    nc.scalar.activation(rz[:, 2 * Bc:4 * Bc],
                                         gA[:, 2 * Bc:4 * Bc], AF.Sigmoid)
                    # off the critical chain (DVE, z-dependent only):
                    # u = z*h, w = 1-z  (NOT on gpsimd/POOL: its tensor ops
                    # silently corrupt on this runtime - sim-only support)
                    nc.vector.tensor_mul(u2[:], z2, hbf_prev[:, hsl])
                    nc.vector.tensor_scalar(w2[:], z2, -1.0, 1.0,
                                            OP.mult, OP.add)
                    nc.vector.tensor_mul(t1[:], r2, gB[:, 2 * Bc:4 * Bc])
                    nc.vector.tensor_add(t2[:], t1[:], gB[:, 0:2 * Bc])
                    nc.scalar.activation(n2[:], t2[:], AF.Tanh)
                    # h' = (1-z)*n + z*h
                    nc.vector.tensor_mul(e2[:], n2[:], w2[:])
                    nc.vector.tensor_add(hbf_next[:, hsl], e2[:], u2[:])
                    continue
                # bias path (step 1 / nonzero biases): per-chunk, per-partition
                # biases differ per chunk so activations stay [128, Bc]
                for ci in range(2):
                    c = 2 * hf + ci
                    csl = slice(c * Bc, (c + 1) * Bc)
                    cc = slice(c, c + 1)
                    rps = gA[:, ci * Bc:(ci + 1) * Bc]
                    zps = gA[:, 2 * Bc + ci * Bc: 2 * Bc + (ci + 1) * Bc]
                    inps = gB[:, ci * Bc:(ci + 1) * Bc]
                    hnps = gB[:, 2 * Bc + ci * Bc: 2 * Bc + (ci + 1) * Bc]
                    r = ew.tile([128, Bc], BF16, tag="r")
                    z = ew.tile([128, Bc], BF16, tag="z")
                    t1 = ew.tile([128, Bc], BF16, tag="t1")
                    n = ew.tile([128, Bc], BF16, tag="n")
                    d = ew.tile([128, Bc], BF16, tag="d")
                    e = ew.tile([128, Bc], BF16, tag="e")
                    if first:
                        nc.scalar.activation(r[:], rps, AF.Sigmoid, bias=b1r[:, cc])
                        nc.scalar.activation(z[:], zps, AF.Sigmoid, bias=b1z[:, cc])
                    elif has_rz:
                        nc.scalar.activation(r[:], rps, AF.Sigmoid,
                                             bias=opt["br"][:, cc])
                        nc.scalar.activation(z[:], zps, AF.Sigmoid,
                                             bias=opt["bz"][:, cc])
                    else:
                        nc.scalar.activation(r[:], rps, AF.Sigmoid)
                        nc.scalar.activation(z[:], zps, AF.Sigmoid)
                    if has_hn:
                        nc.vector.scalar_tensor_tensor(t1[:], hnps,
                                                       opt["bhn"][:, cc],
                                                       r[:], OP.add, OP.mult)
                    else:
                        nc.vector.tensor_mul(t1[:], r[:], hnps)
                    if first:
                        nc.scalar.activation(n[:], t1[:], AF.Tanh, bias=b1n[:, cc])
                    else:
                        t2 = ew.tile([128, Bc], BF16, tag="t2")
                        nc.vector.tensor_add(t2[:], t1[:], inps)
                        if has_in:
                            nc.scalar.activation(n[:], t2[:], AF.Tanh,
                                                 bias=opt["bin"][:, cc])
                        else:
                            nc.scalar.activation(n[:], t2[:], AF.Tanh)
                    nc.vector.scalar_tensor_tensor(d[:], n[:], -1.0,
                                                   hbf_prev[:, csl],
                                                   OP.mult, OP.add)
                    nc.vector.tensor_mul(e[:], z[:], d[:])
                    nc.vector.tensor_add(hbf_next[:, csl], n[:], e[:])
            return hbf_next

        def proj_step(t, hbf):
            # wproj is the stationary operand (128-col zero-padded -> FWL);
            # output lands V-major [V(pad 128), Bc]; host de-transposes.
            pj = psum.tile([128, Bc], F32, tag="proj", bufs=2)
            for k in range(KC):
                nc.tensor.matmul(pj[:], wproj[:, k, :],
                                 hbf[:, k * Bc:(k + 1) * Bc],
                                 start=(k == 0), stop=(k == KC - 1))
            if has_proj:
                nc.vector.tensor_add(logits[0:V, t - 1, :], pj[0:V, :], bproj[:])
            else:
                nc.scalar.copy(logits[0:V, t - 1, :], pj[0:V, :])

        def gru_step_mm(t):
            first = (t == 1)
            for hf in range(2):
                gA = psum.tile([128, 4 * Bc], F32, tag="gA", bufs=3)
                gB = psum.tile([128, 4 * Bc], F32, tag="gB", bufs=3)
                emit_half_mms(first, hf, gA, gB, hbf_cur)

        if mode.startswith("chain"):
            # dependency-chain microbenchmarks: each "step" = 10 dependent ops
            ca = ew.tile([128, Bc], BF16, tag="ca")
            cb = ew.tile([128, Bc], BF16, tag="cb")
            nc.vector.tensor_add(ca[:], featT[:, 0, :], featT[:, 1, :])
            nc.vector.tensor_add(cb[:], featT[:, 1, :], featT[:, 2, :])
            acc = ca
            for t in range(steps * reps):
                for i in range(10):
                    nxt = ew.tile([128, Bc], BF16, tag=f"cc{i % 4}")
                    if mode == "chain_dve" or (mode == "chain_mix" and i % 2 == 0):
                        nc.vector.tensor_add(nxt[:], acc[:], cb[:])
                    else:
                        nc.scalar.activation(nxt[:], acc[:], AF.Sigmoid)
                    acc = nxt
            nc.vector.tensor_add(logits[0:Bc, 0, 0:Bc], acc[0:Bc, 0:Bc],
                                 acc[0:Bc, 0:Bc])
            nc.sync.dma_start(out_d[:], logits[:])
            nc.compile()
            return nc

        for rep in range(reps):
            for t in range(1, steps + 1):
                if mode == "mm":
                    gru_step_mm(t)
                elif mode == "mmproj":
                    gru_step_mm(t)
                    proj_step(t, hbf_cur)
                elif mode == "noproj":
                    hbf_cur = gru_step(t, hbf_cur)
                else:
                    hbf_cur = gru_step(t, hbf_cur)
                    proj_step(t, hbf_cur)
                if mode == "full" and t % 4 == 0 and rep == reps - 1:
                    # stream logits out in 4-step batches (contiguous runs),
                    # hidden under compute instead of a ~28us end tail
                    nc.sync.dma_start(out_d[:, t - 4:t, :],
                                      logits[0:V, t - 4:t, :])
        if mode == "full":
            if steps % 4 != 0:
                t0 = steps - steps % 4
                nc.sync.dma_start(out_d[:, t0:steps, :], logits[0:V, t0:steps, :])
        else:
            nc.sync.dma_start(out_d[:], logits[0:V])

    nc.compile()
    return nc


def _prep_inputs(feat, w_hp, b_hp, embed, w_ih, w_hh, b_ih, b_hh, w_proj, b_proj):
    f32 = np.float32
    feat = np.asarray(feat, f32)
    w_hp = np.asarray(w_hp, f32)
    b_hp = np.asarray(b_hp, f32)
    embed = np.asarray(embed, f32)
    w_ih = np.asarray(w_ih, f32)
    w_hh = np.asarray(w_hh, f32)
    b_ih = np.asarray(b_ih, f32)
    b_hh = np.asarray(b_hh, f32)
    w_proj = np.asarray(w_proj, f32)
    b_proj = np.asarray(b_proj, f32)

    def chunk_bias(v):          # [H] -> [128, KC] (col c = chunk c)
        return np.ascontiguousarray(v.reshape(KC, 128).T.astype(f32))

    Wc = np.concatenate([
        w_ih[0:H] + w_hh[0:H],
        w_ih[H:2 * H] + w_hh[H:2 * H],
        w_ih[2 * H:3 * H],
        w_hh[2 * H:3 * H],
    ], axis=0)                                   # [4H, H]
    wT = np.ascontiguousarray(Wc.T.reshape(KC, 128, 4 * H).astype(FP8_NP))
    whhT = np.ascontiguousarray(w_hh.T.reshape(KC, 128, 3 * H).astype(BF16_NP))
    whpT = np.ascontiguousarray(w_hp.reshape(KF, 128, H).astype(BF16_NP))
    wproj_pad = np.zeros((H, 128), f32)
    wproj_pad[:, 0:V] = w_proj
    wproj = np.ascontiguousarray(wproj_pad.reshape(KC, 128, 128).astype(BF16_NP))

    g0 = w_ih @ embed[SOS] + b_ih               # [3H]
    common = dict(wT=wT, whhT=whhT, whpT=whpT, wproj=wproj,
                  b1r=chunk_bias(g0[0:H] + b_hh[0:H]),
                  b1z=chunk_bias(g0[H:2 * H] + b_hh[H:2 * H]),
                  b1n=chunk_bias(g0[2 * H:3 * H]))

    biases = set()
    if np.any(b_ih[0:2 * H] + b_hh[0:2 * H]):
        biases.add("rz")
        common["br"] = chunk_bias(b_ih[0:H] + b_hh[0:H])
        common["bz"] = chunk_bias(b_ih[H:2 * H] + b_hh[H:2 * H])
    if np.any(b_hh[2 * H:]):
        biases.add("hn")
        common["bhn"] = chunk_bias(b_hh[2 * H:])
    if np.any(b_ih[2 * H:]):
        biases.add("in")
        common["bin"] = chunk_bias(b_ih[2 * H:])
    if np.any(b_hp):
        biases.add("hp")
        common["bhp"] = chunk_bias(b_hp)
    if np.any(b_proj):
        biases.add("proj")
        common["bproj"] = np.ascontiguousarray(
            np.broadcast_to(b_proj[:, None], (V, Bc)).astype(f32))

    featT = feat.T.astype(BF16_NP)               # [FEAT, B]
    in_maps = []
    for c in range(NCORES):
        m = dict(common)
        m["featT"] = np.ascontiguousarray(
            featT[:, c * Bc:(c + 1) * Bc].reshape(KF, 128, Bc))
        in_maps.append(m)
    return frozenset(biases), in_maps


def kernel(**inputs) -> np.ndarray:
    global LAST_RESULTS
    biases, in_maps = _prep_inputs(**inputs)
    if biases not in _PROGRAM_CACHE:
        _PROGRAM_CACHE[biases] = _build(biases)
    nc = _PROGRAM_CACHE[biases]
    res = run_bass_kernel_spmd(nc, in_maps, list(range(NCORES)))
    LAST_RESULTS = res
    outs = [res.results[c]["out"].transpose(2, 0, 1) for c in range(NCORES)]
    out = np.concatenate(outs, axis=0)
    return np.ascontiguousarray(out)

